# revision 4
# baseline (speedup 1.0000x reference)
"""Trainium2 Bass kernel for nn_DecoderLayer (B=8,S=T=512,D=1024,H=16,K=64,F=4096).

Sharding: data-parallel over batch — 8 batch elements onto 8 NeuronCores,
weights replicated, no collectives. Each core computes the full decoder layer
for its batch element.

All matmuls run in bf16 with fp32 PSUM accumulation. Softmax is computed in
natural [S,T] layout (reduction along the free axis via ScalarE accum_out);
normalized probabilities are PE-transposed blockwise to [T,S] to feed the
attn@V matmul. Heads (K=64) are packed two per matmul via tile_position
row/col tiling. Weights are streamed from HBM through small cycling pools.
"""
import sys

if "/opt/trn_rl_repo" not in sys.path:
    sys.path.insert(0, "/opt/trn_rl_repo")

from contextlib import ExitStack

import numpy as np
import ml_dtypes

import concourse.bass as bass  # noqa: F401
from concourse import bacc
import concourse.tile as tile
from concourse import mybir
from concourse.bass_utils import run_bass_kernel_spmd
from concourse.masks import make_identity

F32 = mybir.dt.float32
BF16 = mybir.dt.bfloat16
AF = mybir.ActivationFunctionType
OP = mybir.AluOpType

B, S, T, D, H, K, F = 8, 512, 512, 1024, 16, 64, 4096
HK = H * K          # 1024
ND = D // 128       # 8
NS = S // 128       # 4
NF = F // 128       # 32
NHP = H // 2        # 8
EPS = 1e-6
NEG = -1e9
SCALE = 1.0 / 8.0   # 1/sqrt(K)

_BUILD_CACHE = {}


def _build(flags):
    (use_mask1, qk_bias, v_bias, o_bias, fc1_bias, fc2_bias,
     ln1_triv, ln2_triv, ln3_triv) = flags

    nc = bacc.Bacc("TRN2", target_bir_lowering=False)
    dp = nc.declare_dram_parameter

    dd = {}
    dd["xT"] = dp("xT", [D, S], BF16, isOutput=False)
    dd["xr"] = dp("xr", [S, D], F32, isOutput=False)      # x (+bo1 folded on host)
    dd["encT"] = dp("encT", [D, T], BF16, isOutput=False)
    # pre-tiled weights (host layout):
    for w in ("wq1", "wk1", "wq2", "wk2"):
        dd[w] = dp(w, [ND, D, 128], BF16, isOutput=False)
    for w in ("wv1", "wo1", "wv2", "wo2"):
        dd[w] = dp(w, [2, HK, 512], BF16, isOutput=False)
    dd["wfc1"] = dp("wfc1", [NF, D, 128], BF16, isOutput=False)
    dd["wfc2"] = dp("wfc2", [2, 4, 1024, 512], BF16, isOutput=False)
    if use_mask1:
        dd["maskN"] = dp("maskN", [S, T], F32, isOutput=False)
    if qk_bias:
        dd["bqk"] = dp("bqk", [128, 4 * ND], F32, isOutput=False)
    if v_bias:
        dd["bv_bc"] = dp("bv_bc", [128, 2 * HK], F32, isOutput=False)
    if o_bias:
        dd["bo2_bc"] = dp("bo2_bc", [128, D], F32, isOutput=False)
    if fc1_bias:
        dd["bfc1"] = dp("bfc1", [128, NF], F32, isOutput=False)
    if fc2_bias:
        dd["bfc2_bc"] = dp("bfc2_bc", [128, D], F32, isOutput=False)
    if not (ln1_triv and ln2_triv and ln3_triv):
        dd["lnp_bc"] = dp("lnp_bc", [128, 2 * 3 * D], F32, isOutput=False)
    dd["out"] = dp("out", [S, D], F32, isOutput=True)
    dd["aw1"] = dp("aw1", [H, S, T], F32, isOutput=True)
    dd["aw2"] = dp("aw2", [H, S, T], F32, isOutput=True)

    with tile.TileContext(nc) as tc:
        _emit(nc, tc, dd, flags)
    nc.finalize()
    return nc


def _emit(nc, tc, dd, flags):
    (use_mask1, qk_bias, v_bias, o_bias, fc1_bias, fc2_bias,
     ln1_triv, ln2_triv, ln3_triv) = flags

    es = ExitStack()
    with es:
        psf = es.enter_context(tc.tile_pool(name="psf", bufs=3, space="PSUM"))
        sb = es.enter_context(tc.tile_pool(name="sb", bufs=1))
        wlhs_p = es.enter_context(tc.tile_pool(name="wlhs_p", bufs=6))
        wrhs_p = es.enter_context(tc.tile_pool(name="wrhs_p", bufs=3))
        actT_p = es.enter_context(tc.tile_pool(name="actT_p", bufs=3))
        qkT_p = es.enter_context(tc.tile_pool(name="qkT_p", bufs=2))
        v_p = es.enter_context(tc.tile_pool(name="v_p", bufs=2))
        ctxT_p = es.enter_context(tc.tile_pool(name="ctxT_p", bufs=2))
        big_p = es.enter_context(tc.tile_pool(name="big_p", bufs=2))
        ypool = es.enter_context(tc.tile_pool(name="ypool", bufs=2))
        stat = es.enter_context(tc.tile_pool(name="stat", bufs=8))
        ostage = es.enter_context(tc.tile_pool(name="ostage", bufs=2))

        ident_bf = sb.tile([128, 128], BF16, name="ident_bf")
        make_identity(nc, ident_bf[:])
        ident_f32 = sb.tile([128, 128], F32, name="ident_f32")
        make_identity(nc, ident_f32[:])

        def bigtile(nm):
            return big_p.tile([128, NS, D], F32, name=nm, tag="big")

        maskN = None
        if use_mask1:
            maskN = big_p.tile([128, NS, T], F32, name="maskN", tag="big")
            nc.sync.dma_start(maskN[:], dd["maskN"].rearrange("(i p) t -> p i t", p=128))
        xT = actT_p.tile([128, ND, S], BF16, name="xT", tag="actT")
        nc.sync.dma_start(xT[:], dd["xT"].rearrange("(k p) s -> p k s", p=128))
        xr = bigtile("xr")
        nc.sync.dma_start(xr[:], dd["xr"].rearrange("(i p) d -> p i d", p=128))
        encT = actT_p.tile([128, ND, T], BF16, name="encT", tag="actT")
        nc.sync.dma_start(encT[:], dd["encT"].rearrange("(k p) s -> p k s", p=128))

        bqk = bv_bc = bo2_bc = bfc1 = bfc2_bc = lnp = None
        if qk_bias:
            bqk = sb.tile([128, 4 * ND], F32, name="bqk")
            nc.sync.dma_start(bqk[:], dd["bqk"][:])
        if v_bias:
            bv_bc = sb.tile([128, 2 * HK], F32, name="bv_bc")
            nc.sync.dma_start(bv_bc[:], dd["bv_bc"][:])
        if o_bias:
            bo2_bc = sb.tile([128, D], F32, name="bo2_bc")
            nc.sync.dma_start(bo2_bc[:], dd["bo2_bc"][:])
        if fc1_bias:
            bfc1 = sb.tile([128, NF], F32, name="bfc1")
            nc.sync.dma_start(bfc1[:], dd["bfc1"][:])
        if fc2_bias:
            bfc2_bc = sb.tile([128, D], F32, name="bfc2_bc")
            nc.sync.dma_start(bfc2_bc[:], dd["bfc2_bc"][:])
        if "lnp_bc" in dd:
            lnp = sb.tile([128, 2 * 3 * D], F32, name="lnp")
            nc.sync.dma_start(lnp[:], dd["lnp_bc"][:])

        sums = sb.tile([128, NS, H], F32, name="sums")
        recip = sb.tile([128, NS, H], F32, name="recip")

        # ---------- helpers ----------
        def emit_qk_proj(srcT, wname, dst, bcol):
            for m in range(ND):
                wt = wlhs_p.tile([128, ND, 128], BF16, name=f"w_{wname}_{m}",
                                 tag="wlhs")
                nc.sync.dma_start(wt[:], dd[wname][m].rearrange("(k p) c -> p k c",
                                                                p=128))
                ps = psf.tile([128, 512], F32, name=f"qk_{wname}_{m}", tag="psf")
                for k in range(ND):
                    nc.tensor.matmul(ps[:], wt[:, k, :], srcT[:, k, :],
                                     start=(k == 0), stop=(k == ND - 1))
                if qk_bias:
                    nc.scalar.activation(dst[:, m, :], ps[:], AF.Identity,
                                         bias=bqk[:, bcol + m:bcol + m + 1])
                else:
                    nc.scalar.copy(dst[:, m, :], ps[:])

        def emit_v_proj(srcT, wname, v, bv_off):
            for n in range(2):
                wt = wrhs_p.tile([128, ND, 512], BF16, name=f"w_{wname}_{n}",
                                 tag="wrhs")
                nc.sync.dma_start(wt[:], dd[wname][n].rearrange("(k p) c -> p k c",
                                                                p=128))
                for i in range(NS):
                    ps = psf.tile([128, 512], F32, name=f"v_{wname}_{i}_{n}",
                                  tag="psf")
                    for k in range(ND):
                        nc.tensor.matmul(ps[:], srcT[:, k, 128 * i:128 * i + 128],
                                         wt[:, k, :],
                                         start=(k == 0), stop=(k == ND - 1))
                    if v_bias:
                        nc.vector.tensor_add(
                            v[:, i, 512 * n:512 * n + 512], ps[:],
                            bv_bc[:, bv_off + 512 * n:bv_off + 512 * n + 512])
                    else:
                        nc.scalar.copy(v[:, i, 512 * n:512 * n + 512], ps[:])

        def emit_attention(qT, kT, v, ctxT, aw_d, masked, pools, psums):
            expN_p, ab16_p, atT_p, stg_p = pools
            pst, psc = psums
            for hp in range(NHP):
                ab16 = {}
                for i in range(NS):
                    ee = {}
                    for h in (0, 1):
                        r = slice(64 * h, 64 * h + 64)
                        ps = psf.tile([128, 512], F32, name=f"sn_{hp}_{i}_{h}",
                                      tag="psf")
                        nc.tensor.matmul(ps[:], qT[r, hp, 128 * i:128 * i + 128],
                                         kT[r, hp, :], start=True, stop=True,
                                         tile_position=(64 * h, 0))
                        if masked:
                            nc.vector.tensor_add(ps[:], ps[:], maskN[:, i, :])
                        e = expN_p.tile([128, 512], F32, name=f"e_{hp}_{i}_{h}",
                                        tag="expN")
                        nc.scalar.activation(
                            e[:], ps[:], AF.Exp, scale=SCALE,
                            accum_out=sums[:, i, 2 * hp + h:2 * hp + h + 1])
                        ee[h] = e
                    nc.vector.reciprocal(recip[:, i, 2 * hp:2 * hp + 2],
                                         sums[:, i, 2 * hp:2 * hp + 2])
                    for h in (0, 1):
                        g = 2 * hp + h
                        rc = recip[:, i, g:g + 1]
                        a = ab16_p.tile([128, 512], BF16, name=f"ab_{g}_{i}",
                                        tag="ab16")
                        nc.vector.tensor_scalar_mul(a[:], ee[h][:], rc)
                        ab16[(h, i)] = a
                        fa = stg_p.tile([128, 512], F32, name=f"fa_{g}_{i}",
                                        tag="stg")
                        nc.gpsimd.tensor_scalar_mul(fa[:], ee[h][:], rc)
                        nc.sync.dma_start(aw_d[g, 128 * i:128 * i + 128, :], fa[:])
                cps = psc.tile([128, 512], F32, name=f"ctx_{hp}", tag="ctx")
                for h in (0, 1):
                    g = 2 * hp + h
                    for j in range(NS):
                        tp = pst.tile([128, 512], BF16, name=f"tp_{g}_{j}", tag="tr")
                        for i in range(NS):
                            nc.tensor.transpose(tp[:, 128 * i:128 * i + 128],
                                                ab16[(h, i)][:, 128 * j:128 * j + 128],
                                                ident_bf[:])
                        at = atT_p.tile([128, 512], BF16, name=f"at_{g}_{j}",
                                        tag="atT")
                        nc.vector.tensor_copy(at[:], tp[:])
                        nc.tensor.matmul(cps[64 * h:64 * h + 64, :],
                                         v[:, j, 64 * g:64 * g + 64], at[:],
                                         start=(j == 0), stop=(j == NS - 1),
                                         tile_position=(0, 64 * h))
                nc.scalar.copy(ctxT[:, hp, :], cps[:])

        def emit_ln(y_ap, ysum, dst_ap, ln_triv, lnoff):
            st = stat.tile([128, 8], F32, name="st", tag="st")
            nc.vector.tensor_add(st[:, 0:1], ysum[:, 0:1], ysum[:, 1:2])
            nc.vector.tensor_scalar_mul(st[:, 1:2], st[:, 0:1], 1.0 / D)  # mu
            sq0 = stat.tile([128, 2], F32, name="sq0", tag="sq0")
            for hh in range(2):
                sqt = ypool.tile([128, 512], F32, name=f"sqd_{hh}", tag="sqd")
                nc.scalar.activation(sqt[:], y_ap[:, 512 * hh:512 * hh + 512],
                                     AF.Square, accum_out=sq0[:, hh:hh + 1])
            nc.vector.tensor_add(sq0[:, 0:1], sq0[:, 0:1], sq0[:, 1:2])
            nc.vector.tensor_mul(st[:, 2:3], st[:, 1:2], st[:, 1:2])      # mu^2
            nc.vector.scalar_tensor_tensor(st[:, 3:4], sq0[:, 0:1], 1.0 / D,
                                           st[:, 2:3], op0=OP.mult,
                                           op1=OP.subtract)               # var
            nc.vector.tensor_scalar_add(st[:, 3:4], st[:, 3:4], EPS)
            nc.scalar.sqrt(st[:, 4:5], st[:, 3:4])
            nc.vector.reciprocal(st[:, 5:6], st[:, 4:5])                  # rs
            nc.vector.scalar_tensor_tensor(st[:, 6:7], st[:, 1:2], -1.0,
                                           st[:, 5:6], op0=OP.mult,
                                           op1=OP.mult)                   # -mu*rs
            if ln_triv:
                nc.scalar.activation(dst_ap, y_ap[:], AF.Identity,
                                     bias=st[:, 6:7], scale=st[:, 5:6])
            else:
                z = ypool.tile([128, D], F32, name="zt", tag="y")
                nc.scalar.activation(z[:], y_ap[:], AF.Identity,
                                     bias=st[:, 6:7], scale=st[:, 5:6])
                nc.vector.tensor_mul(z[:], z[:], lnp[:, lnoff:lnoff + D])
                nc.vector.tensor_add(dst_ap, z[:], lnp[:, lnoff + D:lnoff + 2 * D])

        def emit_proj_ln(ctxT, wname, xb, out_sb, ln_triv, lnoff):
            wts = []
            for n in range(2):
                wt = wrhs_p.tile([128, ND, 512], BF16, name=f"w_{wname}_{n}",
                                 tag="wrhs")
                nc.sync.dma_start(wt[:], dd[wname][n].rearrange("(k p) c -> p k c",
                                                                p=128))
                wts.append(wt)
            for i in range(NS):
                y_i = ypool.tile([128, D], F32, name=f"y_{lnoff}_{i}", tag="y")
                ysum = stat.tile([128, 2], F32, name=f"ys_{lnoff}_{i}", tag="ys")
                for n in range(2):
                    ps = psf.tile([128, 512], F32, name=f"pr_{lnoff}_{i}_{n}",
                                  tag="psf")
                    for k in range(ND):
                        nc.tensor.matmul(ps[:], ctxT[:, k, 128 * i:128 * i + 128],
                                         wts[n][:, k, :],
                                         start=(k == 0), stop=(k == ND - 1))
                    nc.vector.scalar_tensor_tensor(
                        y_i[:, 512 * n:512 * n + 512], ps[:], 1.0,
                        xb[:, i, 512 * n:512 * n + 512], op0=OP.mult, op1=OP.add,
                        accum_out=ysum[:, n:n + 1])
                emit_ln(y_i, ysum, out_sb[:, i, :], ln_triv, lnoff)

        def emit_transpose(src, dst, pst):
            """src [128,NS,D] f32 natural -> dst [128,ND,S] bf16 transposed."""
            for i in range(NS):
                for half in range(2):
                    tp = pst.tile([128, 512], F32, name=f"tx_{id(src)}_{i}_{half}",
                                  tag="tr")
                    for kk in range(4):
                        k = 4 * half + kk
                        nc.tensor.transpose(tp[:, 128 * kk:128 * kk + 128],
                                            src[:, i, 128 * k:128 * k + 128],
                                            ident_f32[:])
                    nc.scalar.copy(
                        dst[:, 4 * half:4 * half + 4, 128 * i:128 * i + 128], tp[:])

        # ================= MHA1 =================
        qT1 = qkT_p.tile([128, ND, S], BF16, name="qT1", tag="qkT")
        kT1 = qkT_p.tile([128, ND, T], BF16, name="kT1", tag="qkT")
        v1 = v_p.tile([128, NS, HK], BF16, name="v1", tag="v")
        ctxT1 = ctxT_p.tile([128, NHP, S], BF16, name="ctxT1", tag="ctxT")
        emit_qk_proj(xT, "wq1", qT1, 0)
        emit_qk_proj(xT, "wk1", kT1, ND)
        emit_v_proj(xT, "wv1", v1, 0)

        attn_es = ExitStack()
        with attn_es:
            pst = attn_es.enter_context(tc.tile_pool(name="pst", bufs=2,
                                                     space="PSUM"))
            psc = attn_es.enter_context(tc.tile_pool(name="psc", bufs=2,
                                                     space="PSUM"))
            pools = (
                attn_es.enter_context(tc.tile_pool(name="expN_p", bufs=6)),
                attn_es.enter_context(tc.tile_pool(name="ab16_p", bufs=8)),
                attn_es.enter_context(tc.tile_pool(name="atT_p", bufs=6)),
                attn_es.enter_context(tc.tile_pool(name="stg_p", bufs=3)),
            )
            emit_attention(qT1, kT1, v1, ctxT1, dd["aw1"], use_mask1, pools,
                           (pst, psc))

            out1 = bigtile("out1")
            emit_proj_ln(ctxT1, "wo1", xr, out1, ln1_triv, 0)
            out1T = actT_p.tile([128, ND, S], BF16, name="out1T", tag="actT")
            emit_transpose(out1, out1T, pst)

            # ============ MHA2 QKV ============
            qT2 = qkT_p.tile([128, ND, S], BF16, name="qT2", tag="qkT")
            kT2 = qkT_p.tile([128, ND, T], BF16, name="kT2", tag="qkT")
            v2 = v_p.tile([128, NS, HK], BF16, name="v2", tag="v")
            ctxT2 = ctxT_p.tile([128, NHP, S], BF16, name="ctxT2", tag="ctxT")
            emit_qk_proj(out1T, "wq2", qT2, 2 * ND)
            emit_qk_proj(encT, "wk2", kT2, 3 * ND)
            emit_v_proj(encT, "wv2", v2, HK)

            emit_attention(qT2, kT2, v2, ctxT2, dd["aw2"], False, pools,
                           (pst, psc))

            # ============ proj2 + LN2 ============
            if o_bias:
                xb2 = bigtile("xb2")
                for i in range(NS):
                    nc.vector.tensor_add(xb2[:, i, :], out1[:, i, :], bo2_bc[:])
            else:
                xb2 = out1
            out2 = bigtile("out2")
            emit_proj_ln(ctxT2, "wo2", xb2, out2, ln2_triv, 2 * D)
            out2T = actT_p.tile([128, ND, S], BF16, name="out2T", tag="actT")
            emit_transpose(out2, out2T, pst)

        # ================= FFN =================
        ffn_es = ExitStack()
        with ffn_es:
            psff = ffn_es.enter_context(tc.tile_pool(name="psff", bufs=4,
                                                     space="PSUM"))
            hT_pool = ffn_es.enter_context(tc.tile_pool(name="hT_pool", bufs=1))
            hT = hT_pool.tile([128, NF, S], BF16, name="hT")
            for m in range(NF):
                wt = wlhs_p.tile([128, ND, 128], BF16, name=f"w_fc1_{m}", tag="wlhs")
                nc.sync.dma_start(wt[:], dd["wfc1"][m].rearrange("(k p) c -> p k c",
                                                                 p=128))
                ps = psf.tile([128, 512], F32, name=f"h_{m}", tag="psf")
                for k in range(ND):
                    nc.tensor.matmul(ps[:], wt[:, k, :], out2T[:, k, :],
                                     start=(k == 0), stop=(k == ND - 1))
                if fc1_bias:
                    nc.scalar.activation(hT[:, m, :], ps[:], AF.Relu,
                                         bias=bfc1[:, m:m + 1])
                else:
                    nc.scalar.activation(hT[:, m, :], ps[:], AF.Relu)

            if fc2_bias:
                xb3 = bigtile("xb3")
                for i in range(NS):
                    nc.vector.tensor_add(xb3[:, i, :], out2[:, i, :], bfc2_bc[:])
            else:
                xb3 = out2
            y3 = bigtile("y3")
            ysums = [stat.tile([128, 2], F32, name=f"ys3_{i}", tag="ys")
                     for i in range(NS)]
            for n in range(2):
                pss = [psff.tile([128, 512], F32, name=f"f2_{n}_{i}", tag="psff")
                       for i in range(NS)]
                for q in range(4):
                    wt = wrhs_p.tile([128, ND, 512], BF16, name=f"w_fc2_{n}_{q}",
                                     tag="wrhs")
                    nc.sync.dma_start(wt[:],
                                      dd["wfc2"][n, q].rearrange("(k p) c -> p k c",
                                                                 p=128))
                    for kk in range(ND):
                        for i in range(NS):
                            nc.tensor.matmul(
                                pss[i][:], hT[:, 8 * q + kk, 128 * i:128 * i + 128],
                                wt[:, kk, :],
                                start=(q == 0 and kk == 0),
                                stop=(q == 3 and kk == ND - 1))
                for i in range(NS):
                    nc.vector.scalar_tensor_tensor(
                        y3[:, i, 512 * n:512 * n + 512], pss[i][:], 1.0,
                        xb3[:, i, 512 * n:512 * n + 512], op0=OP.mult, op1=OP.add,
                        accum_out=ysums[i][:, n:n + 1])
            for i in range(NS):
                oz = ostage.tile([128, D], F32, name=f"oz_{i}", tag="oz")
                emit_ln(y3[:, i, :], ysums[i], oz[:], ln3_triv, 4 * D)
                nc.sync.dma_start(dd["out"][128 * i:128 * i + 128, :], oz[:])


def _get_nc(flags):
    if flags not in _BUILD_CACHE:
        _BUILD_CACHE[flags] = _build(flags)
    return _BUILD_CACHE[flags]


def kernel(x, enc_output, look_ahead_mask,
           wq1, bq1, wk1, bk1, wv1, bv1, wo1, bo1,
           wq2, bq2, wk2, bk2, wv2, bv2, wo2, bo2,
           g1, be1, g2, be2, g3, be3,
           w_fc1, b_fc1, w_fc2, b_fc2, **extra):
    f32 = lambda a: np.ascontiguousarray(np.asarray(a), dtype=np.float32)
    x = f32(x)
    enc = f32(enc_output)
    mask = np.asarray(look_ahead_mask).reshape(S, T).astype(bool)
    bq1, bk1, bv1, bo1 = [f32(b).reshape(-1) for b in (bq1, bk1, bv1, bo1)]
    bq2, bk2, bv2, bo2 = [f32(b).reshape(-1) for b in (bq2, bk2, bv2, bo2)]
    b_fc1, b_fc2 = f32(b_fc1).reshape(-1), f32(b_fc2).reshape(-1)
    g1, be1, g2, be2, g3, be3 = [f32(v).reshape(-1) for v in (g1, be1, g2, be2, g3, be3)]

    use_mask1 = not np.all(mask)
    qk_bias = any(np.any(b) for b in (bq1, bk1, bq2, bk2))
    v_bias = bool(np.any(bv1) or np.any(bv2))
    o_bias = bool(np.any(bo2))
    fc1_bias = bool(np.any(b_fc1))
    fc2_bias = bool(np.any(b_fc2))
    ln1_triv = bool(np.all(g1 == 1) and not np.any(be1))
    ln2_triv = bool(np.all(g2 == 1) and not np.any(be2))
    ln3_triv = bool(np.all(g3 == 1) and not np.any(be3))
    flags = (use_mask1, qk_bias, v_bias, o_bias, fc1_bias, fc2_bias,
             ln1_triv, ln2_triv, ln3_triv)
    nc = _get_nc(flags)

    bf = lambda a: np.ascontiguousarray(a, dtype=ml_dtypes.bfloat16)

    def tile_lhs(w):  # [D, M] -> [M/128, D, 128]
        w = f32(w).reshape(w.shape[0] if w.ndim == 2 else D, -1)
        d, m = w.shape
        return bf(w.reshape(d, m // 128, 128).transpose(1, 0, 2))

    def tile_rhs(w):  # [K, N] -> [2, K, N/2]
        d, m = w.shape
        return bf(w.reshape(d, 2, m // 2).transpose(1, 0, 2))

    shared = {
        "wq1": tile_lhs(f32(wq1).reshape(D, HK)),
        "wk1": tile_lhs(f32(wk1).reshape(D, HK)),
        "wq2": tile_lhs(f32(wq2).reshape(D, HK)),
        "wk2": tile_lhs(f32(wk2).reshape(D, HK)),
        "wv1": tile_rhs(f32(wv1).reshape(D, HK)),
        "wo1": tile_rhs(f32(wo1).reshape(HK, D)),
        "wv2": tile_rhs(f32(wv2).reshape(D, HK)),
        "wo2": tile_rhs(f32(wo2).reshape(HK, D)),
        "wfc1": tile_lhs(f32(w_fc1)),
        "wfc2": bf(f32(w_fc2).reshape(4, 1024, 2, 512).transpose(2, 0, 1, 3)),
    }
    if use_mask1:
        shared["maskN"] = np.where(mask, 0.0, NEG).astype(np.float32)
    if qk_bias:
        shared["bqk"] = np.ascontiguousarray(np.concatenate(
            [b.reshape(ND, 128).T for b in (bq1, bk1, bq2, bk2)], axis=1),
            dtype=np.float32)
    if v_bias:
        shared["bv_bc"] = np.ascontiguousarray(np.broadcast_to(
            np.concatenate([bv1, bv2]).reshape(1, 2 * HK), (128, 2 * HK)),
            dtype=np.float32)
    if o_bias:
        shared["bo2_bc"] = np.ascontiguousarray(
            np.broadcast_to(bo2.reshape(1, D), (128, D)), dtype=np.float32)
    if fc1_bias:
        shared["bfc1"] = np.ascontiguousarray(b_fc1.reshape(NF, 128).T,
                                              dtype=np.float32)
    if fc2_bias:
        shared["bfc2_bc"] = np.ascontiguousarray(
            np.broadcast_to(b_fc2.reshape(1, D), (128, D)), dtype=np.float32)
    if not (ln1_triv and ln2_triv and ln3_triv):
        lnp = np.concatenate([g1, be1, g2, be2, g3, be3]).reshape(1, 6 * D)
        shared["lnp_bc"] = np.ascontiguousarray(np.broadcast_to(lnp, (128, 6 * D)),
                                                dtype=np.float32)

    in_maps = []
    for b in range(B):
        m = dict(shared)
        m["xT"] = bf(x[b].T)
        m["xr"] = np.ascontiguousarray(x[b] + bo1.reshape(1, D), dtype=np.float32)
        m["encT"] = bf(enc[b].T)
        in_maps.append(m)

    res = run_bass_kernel_spmd(nc, in_maps, core_ids=list(range(B)))
    dec = np.stack([r["out"] for r in res.results])
    aw1 = np.stack([r["aw1"] for r in res.results])
    aw2 = np.stack([r["aw2"] for r in res.results])
    return dec, aw1, aw2


# revision 6
# speedup vs baseline: 2.3134x; 2.3134x over previous
"""Trainium2 Bass kernel for nn_DecoderLayer (B=8,S=T=512,D=1024,H=16,K=64,F=4096).

Sharding: data-parallel over batch — 8 batch elements onto 8 NeuronCores,
weights replicated, no collectives. Each core computes the full decoder layer
for its batch element.

All matmuls run in bf16 with fp32 PSUM accumulation. Softmax is computed in
natural [S,T] layout (reduction along the free axis via ScalarE accum_out);
normalized probabilities are PE-transposed blockwise to [T,S] to feed the
attn@V matmul. Heads (K=64) are packed two per matmul via tile_position
row/col tiling. Weights are streamed from HBM through small cycling pools.
"""
import sys

if "/opt/trn_rl_repo" not in sys.path:
    sys.path.insert(0, "/opt/trn_rl_repo")

from contextlib import ExitStack

import numpy as np
import ml_dtypes

import concourse.bass as bass  # noqa: F401
from concourse import bacc
import concourse.tile as tile
from concourse import mybir
from concourse.bass_utils import run_bass_kernel_spmd
from concourse.masks import make_identity

F32 = mybir.dt.float32
BF16 = mybir.dt.bfloat16
AF = mybir.ActivationFunctionType
OP = mybir.AluOpType

B, S, T, D, H, K, F = 8, 512, 512, 1024, 16, 64, 4096
HK = H * K          # 1024
ND = D // 128       # 8
NS = S // 128       # 4
NF = F // 128       # 32
NHP = H // 2        # 8
EPS = 1e-6
NEG = -1e9
SCALE = 1.0 / 8.0   # 1/sqrt(K)

_BUILD_CACHE = {}


def _build(flags):
    (use_mask1, qk_bias, v_bias, o_bias, fc1_bias, fc2_bias,
     ln1_triv, ln2_triv, ln3_triv) = flags

    nc = bacc.Bacc("TRN2", target_bir_lowering=False)
    dp = nc.declare_dram_parameter

    dd = {}
    dd["xT"] = dp("xT", [D, S], BF16, isOutput=False)
    dd["xr"] = dp("xr", [S, D], F32, isOutput=False)      # x (+bo1 folded on host)
    dd["encT"] = dp("encT", [D, T], BF16, isOutput=False)
    # pre-tiled weights (host layout):
    for w in ("wq1", "wk1", "wq2", "wk2"):
        dd[w] = dp(w, [ND, D, 128], BF16, isOutput=False)
    for w in ("wv1", "wo1", "wv2", "wo2"):
        dd[w] = dp(w, [2, HK, 512], BF16, isOutput=False)
    dd["wfc1"] = dp("wfc1", [NF, D, 128], BF16, isOutput=False)
    dd["wfc2"] = dp("wfc2", [2, 4, 1024, 512], BF16, isOutput=False)
    if use_mask1:
        dd["maskN"] = dp("maskN", [S, T], F32, isOutput=False)
    if qk_bias:
        dd["bqk"] = dp("bqk", [128, 4 * ND], F32, isOutput=False)
    if v_bias:
        dd["bv_bc"] = dp("bv_bc", [128, 2 * HK], F32, isOutput=False)
    if o_bias:
        dd["bo2_bc"] = dp("bo2_bc", [128, D], F32, isOutput=False)
    if fc1_bias:
        dd["bfc1"] = dp("bfc1", [128, NF], F32, isOutput=False)
    if fc2_bias:
        dd["bfc2_bc"] = dp("bfc2_bc", [128, D], F32, isOutput=False)
    if not (ln1_triv and ln2_triv and ln3_triv):
        dd["lnp_bc"] = dp("lnp_bc", [128, 2 * 3 * D], F32, isOutput=False)
    dd["out"] = dp("out", [S, D], F32, isOutput=True)
    dd["aw1"] = dp("aw1", [H, S, T], F32, isOutput=True)
    dd["aw2"] = dp("aw2", [H, S, T], F32, isOutput=True)

    with tile.TileContext(nc) as tc:
        _emit(nc, tc, dd, flags)
    nc.finalize()
    return nc


def _emit(nc, tc, dd, flags):
    (use_mask1, qk_bias, v_bias, o_bias, fc1_bias, fc2_bias,
     ln1_triv, ln2_triv, ln3_triv) = flags

    es = ExitStack()
    with es:
        psf = es.enter_context(tc.tile_pool(name="psf", bufs=3, space="PSUM"))
        sb = es.enter_context(tc.tile_pool(name="sb", bufs=1))
        wlhs_p = es.enter_context(tc.tile_pool(name="wlhs_p", bufs=6))
        wrhs_p = es.enter_context(tc.tile_pool(name="wrhs_p", bufs=3))
        actT_p = es.enter_context(tc.tile_pool(name="actT_p", bufs=3))
        qkT_p = es.enter_context(tc.tile_pool(name="qkT_p", bufs=2))
        v_p = es.enter_context(tc.tile_pool(name="v_p", bufs=2))
        ctxT_p = es.enter_context(tc.tile_pool(name="ctxT_p", bufs=2))
        big_p = es.enter_context(tc.tile_pool(name="big_p", bufs=2))
        ypool = es.enter_context(tc.tile_pool(name="ypool", bufs=2))
        stat = es.enter_context(tc.tile_pool(name="stat", bufs=8))
        ostage = es.enter_context(tc.tile_pool(name="ostage", bufs=2))

        ident_bf = sb.tile([128, 128], BF16, name="ident_bf")
        make_identity(nc, ident_bf[:])
        ident_f32 = sb.tile([128, 128], F32, name="ident_f32")
        make_identity(nc, ident_f32[:])

        def bigtile(nm):
            return big_p.tile([128, NS, D], F32, name=nm, tag="big")

        maskN = None
        if use_mask1:
            maskN = big_p.tile([128, NS, T], F32, name="maskN", tag="big")
            nc.sync.dma_start(maskN[:], dd["maskN"].rearrange("(i p) t -> p i t", p=128))
        xT = actT_p.tile([128, ND, S], BF16, name="xT", tag="actT")
        nc.sync.dma_start(xT[:], dd["xT"].rearrange("(k p) s -> p k s", p=128))
        xr = bigtile("xr")
        nc.sync.dma_start(xr[:], dd["xr"].rearrange("(i p) d -> p i d", p=128))
        encT = actT_p.tile([128, ND, T], BF16, name="encT", tag="actT")
        nc.sync.dma_start(encT[:], dd["encT"].rearrange("(k p) s -> p k s", p=128))

        bqk = bv_bc = bo2_bc = bfc1 = bfc2_bc = lnp = None
        if qk_bias:
            bqk = sb.tile([128, 4 * ND], F32, name="bqk")
            nc.sync.dma_start(bqk[:], dd["bqk"][:])
        if v_bias:
            bv_bc = sb.tile([128, 2 * HK], F32, name="bv_bc")
            nc.sync.dma_start(bv_bc[:], dd["bv_bc"][:])
        if o_bias:
            bo2_bc = sb.tile([128, D], F32, name="bo2_bc")
            nc.sync.dma_start(bo2_bc[:], dd["bo2_bc"][:])
        if fc1_bias:
            bfc1 = sb.tile([128, NF], F32, name="bfc1")
            nc.sync.dma_start(bfc1[:], dd["bfc1"][:])
        if fc2_bias:
            bfc2_bc = sb.tile([128, D], F32, name="bfc2_bc")
            nc.sync.dma_start(bfc2_bc[:], dd["bfc2_bc"][:])
        if "lnp_bc" in dd:
            lnp = sb.tile([128, 2 * 3 * D], F32, name="lnp")
            nc.sync.dma_start(lnp[:], dd["lnp_bc"][:])

        sums = sb.tile([128, NS, H], F32, name="sums")
        recip = sb.tile([128, NS, H], F32, name="recip")

        # ---------- helpers ----------
        def emit_qk_proj(srcT, wname, dst, bcol):
            for m in range(ND):
                wt = wlhs_p.tile([128, ND, 128], BF16, name=f"w_{wname}_{m}",
                                 tag="wlhs")
                nc.sync.dma_start(wt[:], dd[wname][m].rearrange("(k p) c -> p k c",
                                                                p=128))
                ps = psf.tile([128, 512], F32, name=f"qk_{wname}_{m}", tag="psf")
                for k in range(ND):
                    nc.tensor.matmul(ps[:], wt[:, k, :], srcT[:, k, :],
                                     start=(k == 0), stop=(k == ND - 1))
                if qk_bias:
                    nc.scalar.activation(dst[:, m, :], ps[:], AF.Identity,
                                         bias=bqk[:, bcol + m:bcol + m + 1])
                else:
                    nc.scalar.copy(dst[:, m, :], ps[:])

        def emit_v_proj(srcT, wname, v, bv_off):
            for n in range(2):
                wt = wrhs_p.tile([128, ND, 512], BF16, name=f"w_{wname}_{n}",
                                 tag="wrhs")
                nc.sync.dma_start(wt[:], dd[wname][n].rearrange("(k p) c -> p k c",
                                                                p=128))
                for i in range(NS):
                    ps = psf.tile([128, 512], F32, name=f"v_{wname}_{i}_{n}",
                                  tag="psf")
                    for k in range(ND):
                        nc.tensor.matmul(ps[:], srcT[:, k, 128 * i:128 * i + 128],
                                         wt[:, k, :],
                                         start=(k == 0), stop=(k == ND - 1))
                    if v_bias:
                        nc.vector.tensor_add(
                            v[:, i, 512 * n:512 * n + 512], ps[:],
                            bv_bc[:, bv_off + 512 * n:bv_off + 512 * n + 512])
                    else:
                        nc.scalar.copy(v[:, i, 512 * n:512 * n + 512], ps[:])

        def emit_attention(qT, kT, v, ctxT, aw_d, masked, pools, psums):
            expN_p, ab16_p, atT_p, stg_p = pools
            pst, psc = psums

            def phase_scores(hp):
                ab16 = {}
                for i in range(NS):
                    ee = {}
                    for h in (0, 1):
                        r = slice(64 * h, 64 * h + 64)
                        ps = psf.tile([128, 512], F32, name=f"sn_{hp}_{i}_{h}",
                                      tag="psf")
                        nc.tensor.matmul(ps[:], qT[r, hp, 128 * i:128 * i + 128],
                                         kT[r, hp, :], start=True, stop=True,
                                         tile_position=(64 * h, 0))
                        if masked:
                            nc.vector.tensor_add(ps[:], ps[:], maskN[:, i, :])
                        e = expN_p.tile([128, 512], F32, name=f"e_{hp}_{i}_{h}",
                                        tag="expN")
                        nc.scalar.activation(
                            e[:], ps[:], AF.Exp, scale=SCALE,
                            accum_out=sums[:, i, 2 * hp + h:2 * hp + h + 1])
                        ee[h] = e
                    nc.vector.reciprocal(recip[:, i, 2 * hp:2 * hp + 2],
                                         sums[:, i, 2 * hp:2 * hp + 2])
                    for h in (0, 1):
                        g = 2 * hp + h
                        rc = recip[:, i, g:g + 1]
                        fa = stg_p.tile([128, 512], F32, name=f"fa_{g}_{i}",
                                        tag="stg")
                        nc.vector.scalar_tensor_tensor(
                            fa[:], ee[h][:], rc, ee[h][:],
                            op0=OP.mult, op1=OP.bypass)
                        nc.sync.dma_start(aw_d[g, 128 * i:128 * i + 128, :], fa[:])
                        a = ab16_p.tile([128, 512], BF16, name=f"ab_{g}_{i}",
                                        tag="ab16")
                        nc.vector.tensor_copy(a[:], fa[:])
                        ab16[(h, i)] = a
                return ab16

            def phase_ctx(hp, ab16):
                cps = psc.tile([128, 512], F32, name=f"ctx_{hp}", tag="ctx")
                for h in (0, 1):
                    g = 2 * hp + h
                    for j in range(NS):
                        tp = pst.tile([128, 512], BF16, name=f"tp_{g}_{j}", tag="tr")
                        for i in range(NS):
                            nc.tensor.transpose(tp[:, 128 * i:128 * i + 128],
                                                ab16[(h, i)][:, 128 * j:128 * j + 128],
                                                ident_bf[:])
                        at = atT_p.tile([128, 512], BF16, name=f"at_{g}_{j}",
                                        tag="atT")
                        nc.vector.tensor_copy(at[:], tp[:])
                        nc.tensor.matmul(cps[64 * h:64 * h + 64, :],
                                         v[:, j, 64 * g:64 * g + 64], at[:],
                                         start=(j == 0), stop=(j == NS - 1),
                                         tile_position=(0, 64 * h))
                nc.scalar.copy(ctxT[:, hp, :], cps[:])

            prev = None
            for hp in range(NHP):
                ab = phase_scores(hp)
                if prev is not None:
                    phase_ctx(hp - 1, prev)
                prev = ab
            phase_ctx(NHP - 1, prev)

        def emit_ln(y_ap, ysum, dst_ap, ln_triv, lnoff):
            st = stat.tile([128, 8], F32, name="st", tag="st")
            nc.vector.tensor_add(st[:, 0:1], ysum[:, 0:1], ysum[:, 1:2])
            nc.vector.tensor_scalar_mul(st[:, 1:2], st[:, 0:1], 1.0 / D)  # mu
            sq0 = stat.tile([128, 2], F32, name="sq0", tag="sq0")
            for hh in range(2):
                sqt = ypool.tile([128, 512], F32, name=f"sqd_{hh}", tag="sqd")
                nc.scalar.activation(sqt[:], y_ap[:, 512 * hh:512 * hh + 512],
                                     AF.Square, accum_out=sq0[:, hh:hh + 1])
            nc.vector.tensor_add(sq0[:, 0:1], sq0[:, 0:1], sq0[:, 1:2])
            nc.vector.tensor_mul(st[:, 2:3], st[:, 1:2], st[:, 1:2])      # mu^2
            nc.vector.scalar_tensor_tensor(st[:, 3:4], sq0[:, 0:1], 1.0 / D,
                                           st[:, 2:3], op0=OP.mult,
                                           op1=OP.subtract)               # var
            nc.vector.tensor_scalar_add(st[:, 3:4], st[:, 3:4], EPS)
            nc.scalar.sqrt(st[:, 4:5], st[:, 3:4])
            nc.vector.reciprocal(st[:, 5:6], st[:, 4:5])                  # rs
            nc.vector.scalar_tensor_tensor(st[:, 6:7], st[:, 1:2], -1.0,
                                           st[:, 5:6], op0=OP.mult,
                                           op1=OP.mult)                   # -mu*rs
            if ln_triv:
                nc.scalar.activation(dst_ap, y_ap[:], AF.Identity,
                                     bias=st[:, 6:7], scale=st[:, 5:6])
            else:
                z = ypool.tile([128, D], F32, name="zt", tag="y")
                nc.scalar.activation(z[:], y_ap[:], AF.Identity,
                                     bias=st[:, 6:7], scale=st[:, 5:6])
                nc.vector.tensor_mul(z[:], z[:], lnp[:, lnoff:lnoff + D])
                nc.vector.tensor_add(dst_ap, z[:], lnp[:, lnoff + D:lnoff + 2 * D])

        def emit_proj_ln(ctxT, wname, xb, out_sb, ln_triv, lnoff):
            wts = []
            for n in range(2):
                wt = wrhs_p.tile([128, ND, 512], BF16, name=f"w_{wname}_{n}",
                                 tag="wrhs")
                nc.sync.dma_start(wt[:], dd[wname][n].rearrange("(k p) c -> p k c",
                                                                p=128))
                wts.append(wt)
            for i in range(NS):
                y_i = ypool.tile([128, D], F32, name=f"y_{lnoff}_{i}", tag="y")
                ysum = stat.tile([128, 2], F32, name=f"ys_{lnoff}_{i}", tag="ys")
                for n in range(2):
                    ps = psf.tile([128, 512], F32, name=f"pr_{lnoff}_{i}_{n}",
                                  tag="psf")
                    for k in range(ND):
                        nc.tensor.matmul(ps[:], ctxT[:, k, 128 * i:128 * i + 128],
                                         wts[n][:, k, :],
                                         start=(k == 0), stop=(k == ND - 1))
                    nc.vector.scalar_tensor_tensor(
                        y_i[:, 512 * n:512 * n + 512], ps[:], 1.0,
                        xb[:, i, 512 * n:512 * n + 512], op0=OP.mult, op1=OP.add,
                        accum_out=ysum[:, n:n + 1])
                emit_ln(y_i, ysum, out_sb[:, i, :], ln_triv, lnoff)

        def emit_transpose(src, dst, pst):
            """src [128,NS,D] f32 natural -> dst [128,ND,S] bf16 transposed."""
            for i in range(NS):
                for half in range(2):
                    tp = pst.tile([128, 512], F32, name=f"tx_{id(src)}_{i}_{half}",
                                  tag="tr")
                    for kk in range(4):
                        k = 4 * half + kk
                        nc.tensor.transpose(tp[:, 128 * kk:128 * kk + 128],
                                            src[:, i, 128 * k:128 * k + 128],
                                            ident_f32[:])
                    nc.scalar.copy(
                        dst[:, 4 * half:4 * half + 4, 128 * i:128 * i + 128], tp[:])

        # ================= MHA1 =================
        qT1 = qkT_p.tile([128, ND, S], BF16, name="qT1", tag="qkT")
        kT1 = qkT_p.tile([128, ND, T], BF16, name="kT1", tag="qkT")
        v1 = v_p.tile([128, NS, HK], BF16, name="v1", tag="v")
        ctxT1 = ctxT_p.tile([128, NHP, S], BF16, name="ctxT1", tag="ctxT")
        emit_qk_proj(xT, "wq1", qT1, 0)
        emit_qk_proj(xT, "wk1", kT1, ND)
        emit_v_proj(xT, "wv1", v1, 0)

        attn_es = ExitStack()
        with attn_es:
            pst = attn_es.enter_context(tc.tile_pool(name="pst", bufs=2,
                                                     space="PSUM"))
            psc = attn_es.enter_context(tc.tile_pool(name="psc", bufs=2,
                                                     space="PSUM"))
            pools = (
                attn_es.enter_context(tc.tile_pool(name="expN_p", bufs=6)),
                attn_es.enter_context(tc.tile_pool(name="ab16_p", bufs=16)),
                attn_es.enter_context(tc.tile_pool(name="atT_p", bufs=6)),
                attn_es.enter_context(tc.tile_pool(name="stg_p", bufs=4)),
            )
            emit_attention(qT1, kT1, v1, ctxT1, dd["aw1"], use_mask1, pools,
                           (pst, psc))

            out1 = bigtile("out1")
            emit_proj_ln(ctxT1, "wo1", xr, out1, ln1_triv, 0)
            out1T = actT_p.tile([128, ND, S], BF16, name="out1T", tag="actT")
            emit_transpose(out1, out1T, pst)

            # ============ MHA2 QKV ============
            qT2 = qkT_p.tile([128, ND, S], BF16, name="qT2", tag="qkT")
            kT2 = qkT_p.tile([128, ND, T], BF16, name="kT2", tag="qkT")
            v2 = v_p.tile([128, NS, HK], BF16, name="v2", tag="v")
            ctxT2 = ctxT_p.tile([128, NHP, S], BF16, name="ctxT2", tag="ctxT")
            emit_qk_proj(out1T, "wq2", qT2, 2 * ND)
            emit_qk_proj(encT, "wk2", kT2, 3 * ND)
            emit_v_proj(encT, "wv2", v2, HK)

            emit_attention(qT2, kT2, v2, ctxT2, dd["aw2"], False, pools,
                           (pst, psc))

            # ============ proj2 + LN2 ============
            if o_bias:
                xb2 = bigtile("xb2")
                for i in range(NS):
                    nc.vector.tensor_add(xb2[:, i, :], out1[:, i, :], bo2_bc[:])
            else:
                xb2 = out1
            out2 = bigtile("out2")
            emit_proj_ln(ctxT2, "wo2", xb2, out2, ln2_triv, 2 * D)
            out2T = actT_p.tile([128, ND, S], BF16, name="out2T", tag="actT")
            emit_transpose(out2, out2T, pst)

        # ================= FFN =================
        ffn_es = ExitStack()
        with ffn_es:
            psff = ffn_es.enter_context(tc.tile_pool(name="psff", bufs=4,
                                                     space="PSUM"))
            hT_pool = ffn_es.enter_context(tc.tile_pool(name="hT_pool", bufs=1))
            hT = hT_pool.tile([128, NF, S], BF16, name="hT")
            for m in range(NF):
                wt = wlhs_p.tile([128, ND, 128], BF16, name=f"w_fc1_{m}", tag="wlhs")
                nc.sync.dma_start(wt[:], dd["wfc1"][m].rearrange("(k p) c -> p k c",
                                                                 p=128))
                ps = psf.tile([128, 512], F32, name=f"h_{m}", tag="psf")
                for k in range(ND):
                    nc.tensor.matmul(ps[:], wt[:, k, :], out2T[:, k, :],
                                     start=(k == 0), stop=(k == ND - 1))
                if fc1_bias:
                    nc.scalar.activation(hT[:, m, :], ps[:], AF.Relu,
                                         bias=bfc1[:, m:m + 1])
                else:
                    nc.scalar.activation(hT[:, m, :], ps[:], AF.Relu)

            if fc2_bias:
                xb3 = bigtile("xb3")
                for i in range(NS):
                    nc.vector.tensor_add(xb3[:, i, :], out2[:, i, :], bfc2_bc[:])
            else:
                xb3 = out2
            y3 = bigtile("y3")
            ysums = [stat.tile([128, 2], F32, name=f"ys3_{i}", tag="ys")
                     for i in range(NS)]
            for n in range(2):
                pss = [psff.tile([128, 512], F32, name=f"f2_{n}_{i}", tag="psff")
                       for i in range(NS)]
                for q in range(4):
                    wt = wrhs_p.tile([128, ND, 512], BF16, name=f"w_fc2_{n}_{q}",
                                     tag="wrhs")
                    nc.sync.dma_start(wt[:],
                                      dd["wfc2"][n, q].rearrange("(k p) c -> p k c",
                                                                 p=128))
                    for kk in range(ND):
                        for i in range(NS):
                            nc.tensor.matmul(
                                pss[i][:], hT[:, 8 * q + kk, 128 * i:128 * i + 128],
                                wt[:, kk, :],
                                start=(q == 0 and kk == 0),
                                stop=(q == 3 and kk == ND - 1))
                for i in range(NS):
                    nc.vector.scalar_tensor_tensor(
                        y3[:, i, 512 * n:512 * n + 512], pss[i][:], 1.0,
                        xb3[:, i, 512 * n:512 * n + 512], op0=OP.mult, op1=OP.add,
                        accum_out=ysums[i][:, n:n + 1])
            for i in range(NS):
                oz = ostage.tile([128, D], F32, name=f"oz_{i}", tag="oz")
                emit_ln(y3[:, i, :], ysums[i], oz[:], ln3_triv, 4 * D)
                nc.sync.dma_start(dd["out"][128 * i:128 * i + 128, :], oz[:])


def _get_nc(flags):
    if flags not in _BUILD_CACHE:
        _BUILD_CACHE[flags] = _build(flags)
    return _BUILD_CACHE[flags]


def kernel(x, enc_output, look_ahead_mask,
           wq1, bq1, wk1, bk1, wv1, bv1, wo1, bo1,
           wq2, bq2, wk2, bk2, wv2, bv2, wo2, bo2,
           g1, be1, g2, be2, g3, be3,
           w_fc1, b_fc1, w_fc2, b_fc2, **extra):
    f32 = lambda a: np.ascontiguousarray(np.asarray(a), dtype=np.float32)
    x = f32(x)
    enc = f32(enc_output)
    mask = np.asarray(look_ahead_mask).reshape(S, T).astype(bool)
    bq1, bk1, bv1, bo1 = [f32(b).reshape(-1) for b in (bq1, bk1, bv1, bo1)]
    bq2, bk2, bv2, bo2 = [f32(b).reshape(-1) for b in (bq2, bk2, bv2, bo2)]
    b_fc1, b_fc2 = f32(b_fc1).reshape(-1), f32(b_fc2).reshape(-1)
    g1, be1, g2, be2, g3, be3 = [f32(v).reshape(-1) for v in (g1, be1, g2, be2, g3, be3)]

    use_mask1 = not np.all(mask)
    qk_bias = any(np.any(b) for b in (bq1, bk1, bq2, bk2))
    v_bias = bool(np.any(bv1) or np.any(bv2))
    o_bias = bool(np.any(bo2))
    fc1_bias = bool(np.any(b_fc1))
    fc2_bias = bool(np.any(b_fc2))
    ln1_triv = bool(np.all(g1 == 1) and not np.any(be1))
    ln2_triv = bool(np.all(g2 == 1) and not np.any(be2))
    ln3_triv = bool(np.all(g3 == 1) and not np.any(be3))
    flags = (use_mask1, qk_bias, v_bias, o_bias, fc1_bias, fc2_bias,
             ln1_triv, ln2_triv, ln3_triv)
    nc = _get_nc(flags)

    bf = lambda a: np.ascontiguousarray(a, dtype=ml_dtypes.bfloat16)

    def tile_lhs(w):  # [D, M] -> [M/128, D, 128]
        w = f32(w).reshape(w.shape[0] if w.ndim == 2 else D, -1)
        d, m = w.shape
        return bf(w.reshape(d, m // 128, 128).transpose(1, 0, 2))

    def tile_rhs(w):  # [K, N] -> [2, K, N/2]
        d, m = w.shape
        return bf(w.reshape(d, 2, m // 2).transpose(1, 0, 2))

    shared = {
        "wq1": tile_lhs(f32(wq1).reshape(D, HK)),
        "wk1": tile_lhs(f32(wk1).reshape(D, HK)),
        "wq2": tile_lhs(f32(wq2).reshape(D, HK)),
        "wk2": tile_lhs(f32(wk2).reshape(D, HK)),
        "wv1": tile_rhs(f32(wv1).reshape(D, HK)),
        "wo1": tile_rhs(f32(wo1).reshape(HK, D)),
        "wv2": tile_rhs(f32(wv2).reshape(D, HK)),
        "wo2": tile_rhs(f32(wo2).reshape(HK, D)),
        "wfc1": tile_lhs(f32(w_fc1)),
        "wfc2": bf(f32(w_fc2).reshape(4, 1024, 2, 512).transpose(2, 0, 1, 3)),
    }
    if use_mask1:
        shared["maskN"] = np.where(mask, 0.0, NEG).astype(np.float32)
    if qk_bias:
        shared["bqk"] = np.ascontiguousarray(np.concatenate(
            [b.reshape(ND, 128).T for b in (bq1, bk1, bq2, bk2)], axis=1),
            dtype=np.float32)
    if v_bias:
        shared["bv_bc"] = np.ascontiguousarray(np.broadcast_to(
            np.concatenate([bv1, bv2]).reshape(1, 2 * HK), (128, 2 * HK)),
            dtype=np.float32)
    if o_bias:
        shared["bo2_bc"] = np.ascontiguousarray(
            np.broadcast_to(bo2.reshape(1, D), (128, D)), dtype=np.float32)
    if fc1_bias:
        shared["bfc1"] = np.ascontiguousarray(b_fc1.reshape(NF, 128).T,
                                              dtype=np.float32)
    if fc2_bias:
        shared["bfc2_bc"] = np.ascontiguousarray(
            np.broadcast_to(b_fc2.reshape(1, D), (128, D)), dtype=np.float32)
    if not (ln1_triv and ln2_triv and ln3_triv):
        lnp = np.concatenate([g1, be1, g2, be2, g3, be3]).reshape(1, 6 * D)
        shared["lnp_bc"] = np.ascontiguousarray(np.broadcast_to(lnp, (128, 6 * D)),
                                                dtype=np.float32)

    in_maps = []
    for b in range(B):
        m = dict(shared)
        m["xT"] = bf(x[b].T)
        m["xr"] = np.ascontiguousarray(x[b] + bo1.reshape(1, D), dtype=np.float32)
        m["encT"] = bf(enc[b].T)
        in_maps.append(m)

    res = run_bass_kernel_spmd(nc, in_maps, core_ids=list(range(B)))
    dec = np.stack([r["out"] for r in res.results])
    aw1 = np.stack([r["aw1"] for r in res.results])
    aw2 = np.stack([r["aw2"] for r in res.results])
    return dec, aw1, aw2
